# revision 12
# baseline (speedup 1.0000x reference)
"""Trainium2 Bass kernel for nn_CycleGNN (8-step projected-direction solver).

Contract: kernel(**inputs) takes the FULL unsharded numpy inputs (keyed as in
setup_inputs()) and returns the full output (preds, labels), each
[131072, 8] float32.  Internally shards the 64 graphs across 8 NeuronCores
(8 graphs per core, graphs never interact -> no collectives), runs a Tile
kernel via run_bass_kernel_spmd, and re-assembles on the host.

Device-side design (per core, 8 graphs, 16384 nodes):
 - The 8 graphs are split into two HALVES (graphs 0-3 / 4-7) with fully
   independent per-half tiles; the two halves' instruction streams are
   interleaved so each half's serial DVE/DMA chains hide under the other
   half's PE bursts (mlp -> d-chain -> einsum1 -> einsum2 -> line search
   is one serial chain per half, but the halves share no data).
 - per-node state is "p-major banded" [64, 128] per half:
   graph q owns partitions [16q, 16q+16); node-within-graph (p%16)*128+c.
 - BOTH P and P^T are SBUF-resident in fp8-e3m4 (scaled by 32), so the
   steady-state loop does no HBM traffic for the projection.
 - einsum1 (df = P^T d) and einsum2 (y = P df) run as 4-way column-tiled
   matvecs (4 graphs concurrent in the PE array via tile_position).
 - line search uses the max-ratio form r = y * (1/xs): the per-element
   divide is replaced by one ACT-LUT reciprocal of xs per step, and the
   per-graph min + broadcast runs on tiny gather/broadcast DMAs (no PE).
"""

import numpy as np
import ml_dtypes

import bass_rust
import concourse.bass as bass
import concourse.tile as tile
from concourse import mybir
from concourse.bass_utils import run_bass_kernel_spmd
from concourse.masks import make_identity

F32 = mybir.dt.float32
BF16 = mybir.dt.bfloat16
FP8 = mybir.dt.float8e3
BF = ml_dtypes.bfloat16
F8 = ml_dtypes.float8_e3m4
PSCALE = 32.0    # P and P^T stored as fp8e3 * PSCALE (absmax ~3.8 < 15.5)

B = 64          # graphs
NMAX = 2048     # nodes per graph (equal-size, sorted vals_batch)
F = 512         # projection basis dim
HID = 128
NFEAT = 64
NUM_STEPS = 8
STEP_ALPHA = 5.0
NCORES = 8
GPC = B // NCORES            # graphs per core = 8
NPC = GPC * NMAX             # nodes per core = 16384
NCH = NMAX // 128            # n-chunks per graph = 16
FCH = F // 128               # f-chunks = 4
GPH = GPC // 2               # graphs per half = 4
NPH = GPH * NMAX             # nodes per half = 8192

AX = mybir.AxisListType
OP = mybir.AluOpType
ACT = mybir.ActivationFunctionType

_COMPILED = {}


def _split_sync_waits(nc, maxw=1):
    """Walrus in this container accepts at most one sync wait per
    instruction; split extra waits into preceding engine-local NoOps."""
    ctr = 0
    for f in nc.m.functions:
        for bb in f.blocks:
            insts = bb.instructions
            out = []
            changed = False
            for ins in insts:
                si = ins.sync_info
                waits = list(si.on_wait) if si is not None else []
                if len(waits) > maxw:
                    reg_waits = [w for w in waits if w.wait_reg is not None]
                    imm_waits = [w for w in waits if w.wait_reg is None]
                    nkeep = max(0, maxw - len(reg_waits))
                    keep = imm_waits[:nkeep]
                    extra = imm_waits[nkeep:]
                    for i in range(0, len(extra), maxw):
                        ctr += 1
                        nop = mybir.InstNoOp(name=f"wsplit-{ctr}", ins=[], outs=[])
                        nop.engine = ins.engine
                        nop.sync_info = bass_rust.SyncInfo(
                            on_wait=extra[i : i + maxw], on_update=[]
                        )
                        out.append(nop)
                    ins.sync_info = bass_rust.SyncInfo(
                        on_wait=reg_waits + keep, on_update=list(si.on_update)
                    )
                    changed = True
                out.append(ins)
            if changed:
                bb.instructions = out
    return ctr


def _tau_schedule():
    taus = []
    tau = 0.01
    for _ in range(NUM_STEPS):
        taus.append(tau)
        tau = max(tau * 0.5, 1e-5)
    return taus


def build_nc(num_steps=NUM_STEPS, debug=False):
    nc = bass.Bass()

    # ---------------- I/O ----------------
    P_d = nc.declare_dram_parameter("P", [128, GPC, NCH, F], FP8, isOutput=False)
    PT_d = nc.declare_dram_parameter("PT", [128, GPC, FCH, NMAX], FP8, isOutput=False)
    nfT_d = nc.declare_dram_parameter("nfT", [NFEAT, NPC], BF16, isOutput=False)
    xs0_d = nc.declare_dram_parameter("xs0", [128, 128], F32, isOutput=False)
    xsol_d = nc.declare_dram_parameter("xsol", [128, 128], F32, isOutput=False)
    w1_d = nc.declare_dram_parameter("w1", [NFEAT + 1, HID], BF16, isOutput=False)
    b1_d = nc.declare_dram_parameter("b1", [HID, 1], F32, isOutput=False)
    w2_d = nc.declare_dram_parameter("w2", [HID, 1], BF16, isOutput=False)
    b2_d = nc.declare_dram_parameter("b2", [1, 1], F32, isOutput=False)
    seg_d = nc.declare_dram_parameter("seg", [64, 64], F32, isOutput=False)

    preds_o = nc.declare_dram_parameter("preds", [NUM_STEPS, NPC], F32, isOutput=True)
    if debug:
        d_dbg = nc.declare_dram_parameter("d_dbg", [2, 64, 128], F32, isOutput=True)
        df_dbg = nc.declare_dram_parameter("df_dbg", [2, 128, 4, FCH], F32, isOutput=True)
        y_dbg = nc.declare_dram_parameter("y_dbg", [2, 64, 128], F32, isOutput=True)
        rmin_dbg = nc.declare_dram_parameter("rmin_dbg", [2, 64, 1], F32, isOutput=True)
        ac_dbg = nc.declare_dram_parameter("ac_dbg", [2, 64, 1], F32, isOutput=True)
        a16_dbg = nc.declare_dram_parameter("a16_dbg", [2, 4, 16], F32, isOutput=True)
    # xs snapshot at the START of each step; labels are computed on the host
    xs_o = nc.declare_dram_parameter("xs_o", [NUM_STEPS, 128, 128], F32, isOutput=True)

    taus = _tau_schedule()

    with tile.TileContext(nc) as tc:
        with (
            tc.tile_pool(name="res", bufs=1) as res,            # resident singles
            tc.tile_pool(name="hp", bufs=8) as hp,              # relu'd hidden chunks
            tc.tile_pool(name="st", bufs=1) as st,              # per-half state
            tc.tile_pool(name="sm", bufs=1) as sm,              # small temps
            tc.tile_pool(name="mh_ps", bufs=3, space="PSUM") as mh_ps,
            tc.tile_pool(name="mi_ps", bufs=2, space="PSUM") as mi_ps,
            tc.tile_pool(name="e1_ps", bufs=1, space="PSUM") as e1_ps,
            tc.tile_pool(name="e2_ps", bufs=2, space="PSUM") as e2_ps,
        ):
            # ---------------- constants / residents ----------------
            identb = res.tile([128, 128], BF16, tag="identb")
            make_identity(nc, identb)

            ones16 = res.tile([4, 16], F32, tag="ones16")
            nc.vector.memset(ones16, 1.0)
            seg = res.tile([64, 64], F32, tag="seg")
            nc.sync.dma_start(out=seg, in_=seg_d[:])

            w1 = res.tile([NFEAT + 1, HID], BF16, tag="w1")
            nc.sync.dma_start(out=w1, in_=w1_d[:])
            b1c = res.tile([HID, 1], F32, tag="b1c")
            nc.sync.dma_start(out=b1c, in_=b1_d[:])
            w2 = res.tile([HID, 1], BF16, tag="w2")
            nc.sync.dma_start(out=w2, in_=w2_d[:])
            b2c = res.tile([128, 1], F32, tag="b2c")
            nc.sync.dma_start(
                out=b2c,
                in_=bass.AP(tensor=b2_d, offset=0, ap=[[0, 128], [1, 1]]),
            )

            # per-half mlp moving operand: rows 0..63 node features, row 64 = xs
            rhsx = [res.tile([NFEAT + 1, NPH], BF16, tag=f"rhsx{h}", name="rhsx") for h in (0, 1)]
            for h in (0, 1):
                nc.scalar.dma_start(
                    out=rhsx[h][0:NFEAT, :], in_=nfT_d[:, NPH * h : NPH * (h + 1)]
                )

            # per-half state (all on partitions 0..63)
            xs = [st.tile([64, 128], F32, tag=f"xs{h}", name="xs") for h in (0, 1)]
            xs_inv = [st.tile([64, 128], F32, tag=f"xsi{h}", name="xsi") for h in (0, 1)]
            rterm = [st.tile([64, 128], F32, tag=f"rt{h}", name="rt") for h in (0, 1)]
            pred = [st.tile([64, 128], BF16, tag=f"pred{h}", name="pred") for h in (0, 1)]
            y_pm = [st.tile([64, 128], BF16, tag=f"y{h}", name="ypm") for h in (0, 1)]
            for h in (0, 1):
                nc.gpsimd.dma_start(out=xs[h], in_=xs0_d[64 * h : 64 * h + 64, :])

            # resident P and P^T (fp8 * 32).  Chunked so step-0's small DMAs
            # on the same queues don't wait ~12us behind a monolithic load;
            # PT chunks are emitted inside step 0 (see loop below).
            sbP = res.tile([128, GPC, NCH, F], FP8, tag="sbP")
            sbPT = res.tile([128, GPC, FCH, NMAX], FP8, tag="sbPT")
            for h, eng in ((0, nc.sync), (1, nc.gpsimd)):
                for g4 in range(GPH):
                    g = GPH * h + g4
                    eng.dma_start(out=sbP[:, g], in_=P_d[:, g])

            def emit_pt_load(h, part):
                eng = (nc.sync, nc.gpsimd)[h]
                for g4 in (part,) if part is not None else range(GPH):
                    g = GPH * h + g4
                    eng.dma_start(out=sbPT[:, g], in_=PT_d[:, g])

            # ---- step-0 init per half: xs row into rhsx, xs_inv, rterm ----
            for h in (0, 1):
                xbf = sm.tile([64, 128], BF16, tag=f"xbf{h}", name="xbf")
                nc.vector.tensor_copy(xbf, xs[h])
                nc.scalar.dma_start(
                    out=rhsx[h][NFEAT : NFEAT + 1, :].rearrange(
                        "o (p c) -> o p c", p=64
                    ),
                    in_=xbf,
                )
                nc.vector.reciprocal(out=xs_inv[h], in_=xs[h])
                t0 = taus[0]
                rtt = sm.tile([64, 128], F32, tag=f"rtt{h}", name="rtt")
                nc.vector.tensor_scalar(
                    out=rtt, in0=xs[h], scalar1=float(1.0 / (3.0 * t0)),
                    scalar2=float(1.0 / 3.0), op0=OP.mult, op1=OP.add,
                )
                nc.vector.reciprocal(out=rterm[h], in_=rtt)
                nc.gpsimd.dma_start(
                    out=xs_o[0][64 * h : 64 * h + 64, :], in_=xs[h]
                )

            # =================== helper emitters ===================

            def emit_mlp(s, h, inject=None):
                """MLP for half h of step s: hidden (K=65) + relu evac +
                4-way col-tiled out matvec + pred evac + per-graph scatter.
                PE order: hg0 hg1 o0 hg2 o1 hg3 o2 o3 (out-round r consumes
                hidden group r = chunks {4q+r}).  `inject()` emits extra
                instructions (other half's tiny d-chain) after hg2."""
                prow = sm.tile([128, 2048], BF16, tag=f"prow{h}", name="prow")
                hq = {}
                tog = [0]

                def hgroup(r):
                    for q in range(4):
                        c = 4 * q + r
                        hps = mh_ps.tile([128, 512], F32, tag="mh", name="hps")
                        nc.tensor.matmul(
                            hps, w1, rhsx[h][:, 512 * c : 512 * (c + 1)],
                            start=True, stop=True,
                        )
                        hpos = hp.tile([128, 512], BF16, tag="h", name="hpos")
                        if tog[0] % 2 == 0:
                            nc.vector.tensor_scalar(
                                out=hpos, in0=hps, scalar1=b1c, scalar2=0.0,
                                op0=OP.add, op1=OP.max,
                            )
                        else:
                            nc.scalar.activation(
                                out=hpos, in_=hps, func=ACT.Relu, bias=b1c
                            )
                        tog[0] += 1
                        hq[c] = hpos

                def oround(r):
                    pp = mi_ps.tile([128, 512], F32, tag="mi", name="pp")
                    for q in range(4):
                        nc.tensor.matmul(
                            pp[32 * q : 32 * q + 1, :],
                            w2, hq[4 * q + r],
                            start=True, stop=True,
                            tile_position=(0, 32 * q),
                        )
                    if r % 2 == 0:
                        nc.scalar.activation(
                            out=prow[:, 512 * r : 512 * (r + 1)], in_=pp,
                            func=ACT.Identity, bias=b2c,
                        )
                    else:
                        nc.vector.tensor_scalar(
                            out=prow[:, 512 * r : 512 * (r + 1)], in0=pp,
                            scalar1=b2c, scalar2=None, op0=OP.add,
                        )

                hgroup(0)
                hgroup(1)
                oround(0)
                hgroup(2)
                if inject is not None:
                    inject()
                oround(1)
                hgroup(3)
                oround(2)
                oround(3)
                # per-graph scatter: prow row 32q (2048 nodes of graph q)
                # -> pred p-major band [16q:16q+16, 0:128]
                for q in range(4):
                    eng = (nc.sync, nc.gpsimd)[q % 2]
                    eng.dma_start(
                        out=pred[h][16 * q : 16 * q + 16, :],
                        in_=prow[32 * q : 32 * q + 1, :].rearrange(
                            "o (p c) -> o p c", p=16
                        ),
                    )
                # preds output straight from row staging (flat node order)
                nc.gpsimd.dma_start(
                    out=preds_o[s, NPH * h : NPH * (h + 1)].rearrange(
                        "(q c) -> q c", q=4
                    ),
                    in_=prow.rearrange("(q o) c -> q o c", q=4)[:, 0:1, :],
                )

            def emit_dchain(h):
                """|pred|_1 per graph -> pscale; d_bf = pred*pscale + rterm.
                Emitted via two parts so the PE op (seg matmul) can sit at a
                chosen PE-queue slot."""
                pp_abs = sm.tile([64, 1], F32, tag=f"pabs{h}", name="pabs")
                junk = sm.tile([64, 128], F32, tag=f"junk{h}", name="junk")
                nc.scalar.activation(
                    out=junk, in_=pred[h], func=ACT.Abs, accum_out=pp_abs
                )
                gs = mi_ps.tile([64, 1], F32, tag="mi", name="gs")
                nc.tensor.matmul(gs, seg, pp_abs, start=True, stop=True)
                pscale = sm.tile([64, 1], F32, tag=f"psc{h}", name="psc")
                nc.vector.reciprocal(pscale, gs)
                d_bf = sm.tile([64, 128], BF16, tag=f"dbf{h}", name="dbf")
                nc.vector.scalar_tensor_tensor(
                    out=d_bf, in0=pred[h], scalar=pscale, in1=rterm[h],
                    op0=OP.mult, op1=OP.add,
                )
                return d_bf

            def emit_dT(h, d_bf):
                """d_bf [64,128] -> d_cols [128,64] via PE transpose.
                d_cols column 16*g4+k = d for (graph g4, node chunk k)."""
                dct = mi_ps.tile([128, 64], BF16, tag="mi", name="dct")
                nc.tensor.transpose(dct, d_bf, identb[0:64, 0:64])
                d_cols = sm.tile([128, 64], BF16, tag=f"dc{h}", name="dc")
                nc.vector.tensor_copy(d_cols, dct)
                return d_cols

            def emit_e1(h, d_cols, inject=None):
                """einsum1: dfp row 32*g4 = 32*df[g]  (4-way col-tiled)."""
                dfp = e1_ps.tile([128, F], F32, tag="e1", name="dfp")
                for k in range(NCH):
                    for g4 in range(4):
                        g = GPH * h + g4
                        nc.tensor.matmul(
                            dfp[32 * g4 : 32 * g4 + 1, :],
                            d_cols[:, 16 * g4 + k : 16 * g4 + k + 1],
                            sbP[:, g, k, :],
                            start=(k == 0),
                            stop=(k == NCH - 1),
                            tile_position=(0, 32 * g4),
                        )
                    if k == 3 and inject is not None:
                        inject()
                return dfp

            def emit_df_evac(h, dfp):
                dfstage = sm.tile([128, F], BF16, tag=f"dfs{h}", name="dfs")
                nc.scalar.activation(
                    out=dfstage, in_=dfp, func=ACT.Identity,
                    scale=float(1.0 / PSCALE),
                )
                return dfstage

            def emit_dfT(h, dfstage):
                """dfstage rows 32*g4 -> df_cols[:, g4, k] (true df, bf16)."""
                df_cols = sm.tile([128, 4, FCH], BF16, tag=f"dfc{h}", name="dfc")
                for k in range(FCH):
                    tp = mi_ps.tile([128, 128], BF16, tag="mi", name="tp")
                    nc.tensor.transpose(
                        tp, dfstage[:, 128 * k : 128 * (k + 1)], identb
                    )
                    nc.vector.tensor_copy(
                        df_cols[:, :, k : k + 1],
                        tp.rearrange("p (a b) -> p a b", b=32)[:, :, 0:1],
                    )
                return df_cols

            def emit_e2_j(h, df_cols, j):
                """einsum2 j-chunk: yp row 32*g4 = 32*y[g][512j:512j+512];
                evac (descale, bf16) then scatter into y_pm bands."""
                yp = e2_ps.tile([128, 512], F32, tag="e2", name="yp")
                for k in range(FCH):
                    for g4 in range(4):
                        g = GPH * h + g4
                        nc.tensor.matmul(
                            yp[32 * g4 : 32 * g4 + 1, :],
                            df_cols[:, g4, k : k + 1],
                            sbPT[:, g, k, 512 * j : 512 * (j + 1)],
                            start=(k == 0),
                            stop=(k == FCH - 1),
                            tile_position=(0, 32 * g4),
                        )
                ys = sm.tile([128, 512], BF16, tag=f"ys{h}", name="ys", bufs=2)
                if j % 2 == 0:
                    nc.vector.tensor_scalar(
                        out=ys, in0=yp, scalar1=float(1.0 / PSCALE),
                        scalar2=None, op0=OP.mult,
                    )
                else:
                    nc.scalar.activation(
                        out=ys, in_=yp, func=ACT.Identity,
                        scale=float(1.0 / PSCALE),
                    )
                for g4 in range(4):
                    eng = (nc.sync, nc.gpsimd)[(j + g4) % 2]
                    eng.dma_start(
                        out=y_pm[h][16 * g4 + 4 * j : 16 * g4 + 4 * j + 4, :],
                        in_=ys[32 * g4 : 32 * g4 + 1, :].rearrange(
                            "o (p c) -> o p c", p=4
                        ),
                    )

            def emit_tail(s, h):
                """line search (max-ratio form), per-graph alpha via tiny
                gather/broadcast DMAs, xs update, next-step prep.  No PE."""
                r = sm.tile([64, 128], F32, tag=f"r{h}", name="r")
                nc.vector.tensor_mul(r, y_pm[h], xs_inv[h])
                rmin = sm.tile([64, 1], F32, tag=f"rmin{h}", name="rmin")
                nc.vector.tensor_reduce(out=rmin, in_=r, axis=AX.X, op=OP.min)
                # gather to [4,16] (graph per partition), reduce, alpha
                r4 = sm.tile([4, 16], F32, tag=f"r4{h}", name="r4")
                nc.sync.dma_start(
                    out=r4,
                    in_=rmin.rearrange("(g b) o -> g (b o)", g=4),
                )
                a4 = sm.tile([4, 1], F32, tag=f"a4{h}", name="a4")
                nc.vector.tensor_reduce(out=a4, in_=r4, axis=AX.X, op=OP.min)
                # alpha = 0.995 / max(-rmin, 0.2)   (0.2 <=> step cap 5.0)
                nc.vector.tensor_scalar(
                    out=a4, in0=a4, scalar1=float(-1.0 / 0.995),
                    scalar2=float(0.2 / 0.995), op0=OP.mult, op1=OP.max,
                )
                nc.vector.reciprocal(a4, a4)
                if debug and s == 0:
                    nc.gpsimd.dma_start(out=rmin_dbg[h], in_=rmin)
                a16 = sm.tile([4, 16], F32, tag=f"a16{h}", name="a16")
                nc.vector.tensor_scalar(
                    out=a16, in0=ones16, scalar1=a4, scalar2=None, op0=OP.mult
                )
                if debug and s == 0:
                    nc.gpsimd.dma_start(out=a16_dbg[h], in_=a16)
                acol = sm.tile([64, 1], F32, tag=f"ac{h}", name="ac")
                for g in range(4):
                    eng = (nc.sync, nc.gpsimd)[g % 2]
                    eng.dma_start(
                        out=acol[16 * g : 16 * g + 16, :],
                        in_=a16[g : g + 1, :].rearrange("o (p c) -> o p c", p=16),
                    )
                if debug and s == 0:
                    nc.gpsimd.dma_start(out=ac_dbg[h], in_=acol)
                # xs += acol * y
                nc.vector.scalar_tensor_tensor(
                    out=xs[h], in0=y_pm[h], scalar=acol, in1=xs[h],
                    op0=OP.mult, op1=OP.add,
                )
                if s + 1 < num_steps:
                    xbf = sm.tile([64, 128], BF16, tag=f"xbf{h}", name="xbf")
                    nc.vector.tensor_copy(xbf, xs[h])
                    nc.scalar.dma_start(
                        out=rhsx[h][NFEAT : NFEAT + 1, :].rearrange(
                            "o (p c) -> o p c", p=64
                        ),
                        in_=xbf,
                    )
                    nc.vector.reciprocal(out=xs_inv[h], in_=xs[h])
                    t1 = taus[s + 1]
                    rtt = sm.tile([64, 128], F32, tag=f"rtt{h}", name="rtt")
                    nc.vector.tensor_scalar(
                        out=rtt, in0=xs[h], scalar1=float(1.0 / (3.0 * t1)),
                        scalar2=float(1.0 / 3.0), op0=OP.mult, op1=OP.add,
                    )
                    nc.vector.reciprocal(out=rterm[h], in_=rtt)
                    nc.gpsimd.dma_start(
                        out=xs_o[s + 1][64 * h : 64 * h + 64, :], in_=xs[h]
                    )

            # =================== the step loop ===================
            dchain_res = {}
            for s in range(num_steps):
                emit_mlp(s, 0)
                if s == 0:
                    emit_pt_load(0, 0)
                    emit_pt_load(0, 1)

                def inj_d0():
                    dchain_res[0] = emit_dchain(0)

                emit_mlp(s, 1, inject=inj_d0)
                if s == 0:
                    emit_pt_load(0, 2)
                    emit_pt_load(0, 3)
                    emit_pt_load(1, None)

                if debug and s == 0:
                    nc.gpsimd.dma_start(out=d_dbg[0], in_=dchain_res[0])
                dc0 = emit_dT(0, dchain_res[0])

                def inj_d1():
                    dchain_res[1] = emit_dchain(1)

                dfp0 = emit_e1(0, dc0, inject=inj_d1)
                dfs0 = emit_df_evac(0, dfp0)

                if debug and s == 0:
                    nc.gpsimd.dma_start(out=d_dbg[1], in_=dchain_res[1])
                dc1 = emit_dT(1, dchain_res[1])
                dfc0 = emit_dfT(0, dfs0)
                if debug and s == 0:
                    nc.gpsimd.dma_start(out=df_dbg[0], in_=dfc0)
                dfp1 = emit_e1(1, dc1)
                dfs1 = emit_df_evac(1, dfp1)

                emit_e2_j(0, dfc0, 0)
                dfc1 = emit_dfT(1, dfs1)
                if debug and s == 0:
                    nc.gpsimd.dma_start(out=df_dbg[1], in_=dfc1)
                for j in range(1, 4):
                    emit_e2_j(0, dfc0, j)
                if debug and s == 0:
                    nc.gpsimd.dma_start(out=y_dbg[0], in_=y_pm[0])
                emit_tail(s, 0)

                for j in range(4):
                    emit_e2_j(1, dfc1, j)
                if debug and s == 0:
                    nc.gpsimd.dma_start(out=y_dbg[1], in_=y_pm[1])
                emit_tail(s, 1)

    _split_sync_waits(nc, maxw=1)
    return nc


def _seg_mat():
    seg = np.zeros((64, 64), np.float32)
    for g in range(4):
        seg[16 * g : 16 * g + 16, 16 * g : 16 * g + 16] = 1.0
    return seg


def _prep_core_inputs(core, proj, x_start, x_solution, node_feat, W1, b1, W2, b2):
    g0 = core * GPC
    n0 = core * NPC
    Pc = proj[g0 : g0 + GPC]  # [8, 2048, 512] f32
    P_f8 = np.ascontiguousarray(
        Pc.reshape(GPC, NCH, 128, F).transpose(2, 0, 1, 3) * PSCALE
    ).astype(F8)
    PT_f8 = np.ascontiguousarray(
        (Pc * PSCALE).transpose(0, 2, 1).reshape(GPC, FCH, 128, NMAX)
        .transpose(2, 0, 1, 3)
    ).astype(F8)
    nfT = np.ascontiguousarray(node_feat[n0 : n0 + NPC].T).astype(BF)
    return {
        "P": P_f8,
        "PT": PT_f8,
        "nfT": nfT,
        "xs0": x_start[n0 : n0 + NPC].reshape(128, 128).astype(np.float32),
        "xsol": x_solution[n0 : n0 + NPC].reshape(128, 128).astype(np.float32),
        "w1": W1.astype(BF),
        "b1": b1.reshape(HID, 1).astype(np.float32),
        "w2": W2.reshape(HID, 1).astype(BF),
        "b2": b2.reshape(1, 1).astype(np.float32),
        "seg": _seg_mat(),
    }


def _numpy_fallback(x_start, x_solution, node_feat, proj_matrix, W1, b1, W2, b2, batch):
    """General (ragged) reference implementation in numpy, used only if
    vals_batch is not the expected equal-size pattern."""
    nb = proj_matrix.shape[0]
    batch = batch.astype(np.int64)
    counts = np.bincount(batch, minlength=nb)
    offsets = np.cumsum(counts) - counts
    pos = np.arange(batch.shape[0]) - offsets[batch]

    def l1norm(x):
        s = np.zeros(nb, x.dtype)
        np.add.at(s, batch, np.abs(x))
        return x / np.clip(s, 1e-8, None)[batch]

    def to_dense(x):
        dense = np.zeros((nb, NMAX), x.dtype)
        m = pos < NMAX
        dense[batch[m], pos[m]] = x[m]
        return dense

    def line_search(x, dvec):
        neg = dvec < 0
        step = np.where(neg, x / np.where(neg, -dvec, 1.0), STEP_ALPHA)
        a = np.full(nb, np.inf, step.dtype)
        np.minimum.at(a, batch, step)
        return np.minimum(a, STEP_ALPHA)[batch]

    def gnn(x):
        h = np.concatenate([node_feat, x[:, None]], axis=-1)
        h = np.maximum(h @ W1 + b1, 0.0)
        return (h @ W2 + b2)[:, 0]

    tau = 0.01
    xs = x_start.astype(np.float32)
    preds, labels = [], []
    for _ in range(NUM_STEPS):
        pred = gnn(xs)
        preds.append(pred)
        labels.append(l1norm(x_solution - xs))
        p = l1norm(pred)
        direction = p + 3.0 * tau / (xs + tau)
        tau = max(tau * 0.5, 1e-5)
        d_dense = to_dense(direction)
        df = np.einsum("bnf,bn->bf", proj_matrix, d_dense)
        proj_dense = np.einsum("bnf,bf->bn", proj_matrix, df)
        proj_flat = proj_dense[batch, np.minimum(pos, NMAX - 1)]
        proj_flat = np.where(pos < NMAX, proj_flat, 0.0)
        alpha = line_search(xs, proj_flat) * 0.995
        xs = xs + alpha * proj_flat
    return np.stack(preds, 1).astype(np.float32), np.stack(labels, 1).astype(np.float32)


def run_on_hw(inputs_list):
    if "plain" not in _COMPILED:
        _COMPILED["plain"] = build_nc()
    nc = _COMPILED["plain"]
    return run_bass_kernel_spmd(nc, inputs_list, list(range(NCORES))).results


def kernel(x_start, x_solution, node_feat, proj_matrix, W1, b1, W2, b2, vals_batch):
    expected = np.repeat(np.arange(B, dtype=np.int64), NMAX)
    vb = np.asarray(vals_batch)
    if vb.shape != expected.shape or not np.array_equal(
        vb.astype(np.int64), expected
    ):
        return _numpy_fallback(
            np.asarray(x_start, np.float32),
            np.asarray(x_solution, np.float32),
            np.asarray(node_feat, np.float32),
            np.asarray(proj_matrix, np.float32),
            np.asarray(W1, np.float32),
            np.asarray(b1, np.float32),
            np.asarray(W2, np.float32),
            np.asarray(b2, np.float32),
            vb,
        )

    x_start = np.asarray(x_start, np.float32)
    x_solution = np.asarray(x_solution, np.float32)
    node_feat = np.asarray(node_feat, np.float32)
    proj_matrix = np.asarray(proj_matrix, np.float32)
    W1 = np.asarray(W1, np.float32)
    b1 = np.asarray(b1, np.float32)
    W2 = np.asarray(W2, np.float32)
    b2 = np.asarray(b2, np.float32)

    ins = [
        _prep_core_inputs(c, proj_matrix, x_start, x_solution, node_feat, W1, b1, W2, b2)
        for c in range(NCORES)
    ]
    results = run_on_hw(ins)
    preds = np.concatenate(
        [results[c]["preds"].T for c in range(NCORES)], axis=0
    ).astype(np.float32)
    # labels = l1norm(x_solution - xs_s) from the per-step xs snapshots
    xs_all = np.concatenate(
        [results[c]["xs_o"].reshape(NUM_STEPS, NPC) for c in range(NCORES)], axis=1
    )  # [NUM_STEPS, TOTAL]
    diff = x_solution[None, :] - xs_all
    d3 = diff.reshape(NUM_STEPS, B, NMAX)
    sums = np.clip(np.abs(d3).sum(axis=2, keepdims=True), 1e-8, None)
    labels = np.ascontiguousarray(
        (d3 / sums).reshape(NUM_STEPS, B * NMAX).T
    ).astype(np.float32)
    return preds, labels
